# revision 6
# baseline (speedup 1.0000x reference)
"""ArcFace head (B=1024, D=512, C=100000) on 8 TRN2 NeuronCores.

Sharding: tensor-parallel along the num_classes axis (partial-FC ArcFace).
Each core holds a [D, C/8] slice of the (pre-normalized, pre-transposed)
weight and computes its [B, C/8] slice of S * cosine via a bf16 TensorE
matmul with fp32 PSUM accumulation. Embeddings (normalized, scaled by S,
transposed to [D, B]) are broadcast to all cores. The per-row additive
angular margin touches exactly B elements of the [B, C] output, so it is
applied on the host after the gather (exact trig identity:
cos(theta + m) = c*cos(m) - sqrt(1-c^2)*sin(m)).

Head/tail scheduling (driven by per-run NTFF trace analysis; mid-stream
already runs gapless at the 216ns/MM warm-PE roofline, so all the
recoverable time is at the kernel edges):

- The DGE queue interleaves descriptors of all in-flight DMAs, so only
  the FIRST DMA on a queue gets an early completion semaphore. The data
  the first matmuls need — w[k0, :512] plus the k0 embedding slice — is
  therefore prepacked on the host into one contiguous [w_k | emb_k] row
  per k-slice ("headbuf"), so one DMA (sync queue, first) carries the
  whole k0 working set with 2KB descriptors. k1 rides first on the
  Scalar queue; k2/k3 follow on sync. Embedding reads for the entire
  kernel come from these packed tiles.
- The first supertile runs k-OUTER (all m-tiles per k-slice), first
  m0-3 then m4-7, so the first 4 matmuls need only 256KB in flight.
- Dummy warm-up matmuls bridge the preamble-to-data window: any PE-idle
  gap resets the HAM activity window and postpones the 1.2->2.4GHz
  un-throttle by a full 3.4us.
- The last tile flushes per-m (128KB) alternating Sync/Scalar triggers,
  and the final row-block is split in two 64KB halves across both
  queues: that transfer is on the exit-barrier critical path.
"""

import os

import numpy as np
import ml_dtypes

import concourse.bass as bass
import concourse.mybir as mybir
from concourse import bacc
from concourse.tile import TileContext
from concourse.bass import ts
from concourse.bass_utils import run_bass_kernel_spmd

# Problem constants (hardcoded per spec)
B, D, C = 1024, 512, 100000
NCORES = 8
CS = C // NCORES          # 12500 classes per core
S, MARGIN, EPS = 30.0, 0.5, 1e-7

P = 128                   # partitions
KS = D // P               # 4 k-subtiles
MS = B // P               # 8 m-subtiles
NT = 512                  # n tile (one PSUM bank of fp32)
# the device computes the largest NT-aligned prefix of each core's CS columns;
# the ragged remainder (212 columns/core, 1.7% of the FLOPs) is computed on
# the host in fp32 — it would otherwise cost inefficient 424B-descriptor DMAs
# and a partial-width matmul pass
DEV_CS = (CS // NT) * NT  # 12288
REM = CS - DEV_CS         # 212
HW = NT + B               # packed head row: [w_k(512) | emb_k(1024)]

BF16 = mybir.dt.bfloat16
_bf16_np = ml_dtypes.bfloat16


def build_nc(warmup_mms=52):
    nc = bacc.Bacc(None, target_bir_lowering=False)
    headbuf = nc.dram_tensor("headbuf", [D, HW], BF16, kind="ExternalInput")
    wT = nc.dram_tensor("wT", [D, DEV_CS], BF16, kind="ExternalInput")
    out = nc.dram_tensor("out", [B, DEV_CS], BF16, kind="ExternalOutput")

    with TileContext(nc) as tc:
        with (
            tc.tile_pool(name="emb", bufs=1) as epool,
            tc.tile_pool(name="w", bufs=4) as wpool,
            tc.tile_pool(name="o", bufs=5) as opool,
            tc.tile_pool(name="ps", bufs=8, space="PSUM") as pspool,
        ):
            headbuf_r = headbuf[:].rearrange("(ko p) x -> p ko x", p=P)
            wT_r = wT[:].rearrange("(ko p) c -> p ko c", p=P)
            out_r = out[:].rearrange("(mo p) c -> p mo c", p=P)

            # PE warm-up: dummy matmuls run during the initial DMA wait so the
            # HAM clock gate is at 2.4 GHz when the real MM stream starts.
            dummy = epool.tile([P, 64], BF16, tag="dummy")
            nc.vector.memset(dummy[:], 0.0)
            wps = pspool.tile([P, NT], mybir.dt.float32, tag="ps")
            for _ in range(warmup_mms):
                nc.tensor.matmul(
                    wps[:64, :64], lhsT=dummy[:64, :], rhs=dummy[:64, :],
                    start=True, stop=True,
                )

            # packed head tile: head[:, k, 0:NT] = w[k, first 512 cols],
            # head[:, k, NT:NT+B] = emb k-slice (used by ALL supertiles)
            head = epool.tile([P, KS, HW], BF16, tag="head", name="head")
            # k0 working set first on sync (split so the first 4 matmuls
            # need only 256KB); k1 first on scalar; k2/k3 follow on sync.
            nc.sync.dma_start(out=head[:, 0, 0:1024], in_=headbuf_r[:, 0, 0:1024])
            nc.scalar.dma_start(out=head[:, 1, :], in_=headbuf_r[:, 1, :])
            nc.sync.dma_start(out=head[:, 0, 1024:HW], in_=headbuf_r[:, 0, 1024:HW])
            nc.sync.dma_start(out=head[:, 2, :], in_=headbuf_r[:, 2, :])
            nc.sync.dma_start(out=head[:, 3, :], in_=headbuf_r[:, 3, :])
            # h1 weights of the first supertile (cols 512:1024), one DMA
            w_first = wpool.tile([P, KS, 2 * NT], BF16, tag="w", name="w_first")
            nc.scalar.dma_start(
                out=w_first[:, :, NT : 2 * NT], in_=wT_r[:, :, NT : 2 * NT]
            )

            def emb(k, m):
                return head[:, k, NT + m * P : NT + (m + 1) * P]

            supers = (
                [(i * 2 * NT, 2 * NT) for i in range(11)]
                + [(11 * 2 * NT, NT), (11 * 2 * NT + NT, NT)]
            )

            # ---- supertile 0: k-outer so matmuls start on partial data ----
            n0, nw = supers[0]
            o_sb = opool.tile([P, MS, 2 * NT], BF16, tag="o")
            for h in range(2):
                h0 = h * NT
                ps_h = [
                    pspool.tile(
                        [P, NT], mybir.dt.float32, tag="ps", name=f"ps_{n0}_{h}_{m}"
                    )
                    for m in range(MS)
                ]
                for mhalf in range(2):
                    for k in range(KS):
                        for m in range(mhalf * MS // 2, (mhalf + 1) * MS // 2):
                            rhs = (
                                head[:, k, h0 : h0 + NT]
                                if h == 0
                                else w_first[:, k, h0 : h0 + NT]
                            )
                            nc.tensor.matmul(
                                ps_h[m][:, :],
                                lhsT=emb(k, m),
                                rhs=rhs,
                                start=(k == 0),
                                stop=(k == KS - 1),
                            )
                for m in range(MS):
                    if m % 2 == 0:
                        nc.scalar.copy(
                            out=o_sb[:, m, h0 : h0 + NT], in_=ps_h[m][:, :]
                        )
                    else:
                        nc.vector.tensor_copy(
                            out=o_sb[:, m, h0 : h0 + NT], in_=ps_h[m][:, :]
                        )
            nc.sync.dma_start(
                out=out_r[:, 0 : MS // 2, n0 : n0 + nw],
                in_=o_sb[:, 0 : MS // 2, :nw],
            )
            nc.sync.dma_start(
                out=out_r[:, MS // 2 : MS, n0 : n0 + nw],
                in_=o_sb[:, MS // 2 : MS, :nw],
            )

            # ---- supertiles 1..12: proven m-outer/k-inner steady state ----
            for idx, (n0, nw) in enumerate(supers[1:], start=1):
                last_tile = idx == len(supers) - 1
                w_sb = wpool.tile([P, KS, 2 * NT], BF16, tag="w", name=f"w_{n0}")
                nc.sync.dma_start(out=w_sb[:, :, :nw], in_=wT_r[:, :, n0 : n0 + nw])
                o_sb = opool.tile([P, MS, 2 * NT], BF16, tag="o")
                for h in range(2):
                    h0 = h * NT
                    hw = min(NT, nw - h0)
                    if hw <= 0:
                        continue
                    for m in range(MS):
                        ps = pspool.tile(
                            [P, NT], mybir.dt.float32, tag="ps", name=f"ps_{n0}_{h}_{m}"
                        )
                        for k in range(KS):
                            nc.tensor.matmul(
                                ps[:, :hw],
                                lhsT=emb(k, m),
                                rhs=w_sb[:, k, h0 : h0 + hw],
                                start=(k == 0),
                                stop=(k == KS - 1),
                            )
                        # split PSUM->SBUF cast copies between ACT and DVE
                        if m % 2 == 0:
                            nc.scalar.copy(
                                out=o_sb[:, m, h0 : h0 + hw], in_=ps[:, :hw]
                            )
                        else:
                            nc.vector.tensor_copy(
                                out=o_sb[:, m, h0 : h0 + hw], in_=ps[:, :hw]
                            )
                        last_h = (h == 1) or (nw <= NT)
                        if last_h and last_tile:
                            if m == MS - 1:
                                # final flush: two 64KB halves on the two
                                # queues in parallel — this transfer is on
                                # the exit-barrier critical path
                                half = nw // 2
                                nc.scalar.dma_start(
                                    out=out_r[:, m : m + 1, n0 : n0 + half],
                                    in_=o_sb[:, m : m + 1, :half],
                                )
                                nc.sync.dma_start(
                                    out=out_r[:, m : m + 1, n0 + half : n0 + nw],
                                    in_=o_sb[:, m : m + 1, half:nw],
                                )
                            else:
                                eng = nc.scalar if m % 2 == 0 else nc.sync
                                eng.dma_start(
                                    out=out_r[:, m : m + 1, n0 : n0 + nw],
                                    in_=o_sb[:, m : m + 1, :nw],
                                )
                        elif not last_tile and last_h and m == MS // 2 - 1:
                            nc.sync.dma_start(
                                out=out_r[:, 0 : MS // 2, n0 : n0 + nw],
                                in_=o_sb[:, 0 : MS // 2, :nw],
                            )
                        elif not last_tile and last_h and m == MS - 1:
                            nc.sync.dma_start(
                                out=out_r[:, MS // 2 : MS, n0 : n0 + nw],
                                in_=o_sb[:, MS // 2 : MS, :nw],
                            )
    nc.finalize()
    return nc


_NC_CACHE = []


def _get_nc():
    if not _NC_CACHE:
        _NC_CACHE.append(build_nc())
    return _NC_CACHE[0]


def _prep_in_maps(embeddings, weight):
    # normalize on host (fp32), fold the ArcFace scale S into the embeddings
    en = embeddings / np.maximum(
        np.linalg.norm(embeddings, axis=1, keepdims=True), 1e-12
    )
    wn = weight / np.maximum(np.linalg.norm(weight, axis=1, keepdims=True), 1e-12)
    embT = np.ascontiguousarray((S * en).T).astype(_bf16_np)  # [D, B]
    wTn = wn.T  # [D, C] view
    in_maps = []
    for i in range(NCORES):
        shard = np.ascontiguousarray(
            wTn[:, i * CS : i * CS + DEV_CS]
        ).astype(_bf16_np)
        headbuf = np.empty((D, HW), dtype=_bf16_np)
        headbuf[:, :NT] = shard[:, :NT]
        headbuf[:, NT:] = embT
        in_maps.append({"headbuf": headbuf, "wT": shard})
    return in_maps, en, wn


def run_device(embeddings, weight, **spmd_kwargs):
    """Runs the device part; returns (full S*cosine [B, C] fp32, raw results)."""
    if not spmd_kwargs.get("trace"):
        # the axon NTFF-profile hook may be absent in this image; make sure an
        # ambient BASS_TRACE env var can't route us onto that path
        os.environ.setdefault("BASS_NEVER_TRACE", "1")
    nc = _get_nc()
    in_maps, en, wn = _prep_in_maps(embeddings, weight)
    try:
        res = run_bass_kernel_spmd(
            nc, in_maps, core_ids=list(range(NCORES)), **spmd_kwargs
        )
    except Exception:
        # rare transient NRT_EXEC_UNIT_UNRECOVERABLE faults have been observed
        # on this fleet (~2 in 12 runs, uncorrelated with kernel structure);
        # one retry costs nothing if the fault persists
        res = run_bass_kernel_spmd(
            nc, in_maps, core_ids=list(range(NCORES)), **spmd_kwargs
        )
    # ragged remainder columns (212 per core) in fp32 on the host
    rem_w = np.concatenate(
        [wn[i * CS + DEV_CS : (i + 1) * CS] for i in range(NCORES)], axis=0
    )  # [NCORES*REM, D]
    rem_out = (S * en) @ rem_w.T  # [B, NCORES*REM]
    out = np.empty((B, C), dtype=np.float32)
    for i in range(NCORES):
        out[:, i * CS : i * CS + DEV_CS] = np.asarray(
            res.results[i]["out"]
        ).astype(np.float32)
        out[:, i * CS + DEV_CS : (i + 1) * CS] = rem_out[
            :, i * REM : (i + 1) * REM
        ]
    return out, res


def apply_margin(out, labels):
    rows = np.arange(B)
    lab = np.asarray(labels).astype(np.int64)
    c = np.clip(out[rows, lab] / S, -1.0 + EPS, 1.0 - EPS)
    out[rows, lab] = S * (c * np.cos(MARGIN) - np.sqrt(1.0 - c * c) * np.sin(MARGIN))
    return out


def kernel(embeddings, weight, labels):
    embeddings = np.asarray(embeddings, dtype=np.float32)
    weight = np.asarray(weight, dtype=np.float32)
    out, _ = run_device(embeddings, weight)
    return apply_margin(out, labels)
